# revision 1
# baseline (speedup 1.0000x reference)
"""Trainium2 Bass kernel for the spiking-dense first-crossing problem.

Computes out[n,y] = min(1 + argmax_t(V[t,n,y] > 1), 64) where
V[t] = (spike mask up to t) @ weight, via one big masked matmul:

  V^T[(y), (n,t)] = W_slice^T @ mask   (W stationary, y on PSUM partitions)

with the {0,1} mask built on-chip from spike times by DVE broadcast
compares, and the first-crossing extracted with ACT sign + DVE
multiply-by-(T - t_idx) + reduce_max.

Sharding: 2-way over Y (output cols) x 4-way over batch N across the 8
NeuronCores; each core computes a (1024 y, 16 n) block of out^T. The
full weight column-slice (2048 x 1024) stays resident in SBUF.
"""
import os
import sys
import numpy as np

for _p in ('/opt/trn_rl_repo',):
    if os.path.isdir(_p) and _p not in sys.path:
        sys.path.append(_p)

X, T, NN, YY = 2048, 64, 64, 2048
Y_SH, N_SH = 2, 4
YC = YY // Y_SH          # 1024 y-cols per core
NCB = NN // N_SH         # 16 batch rows per core
KC = X // 128            # 16 contraction chunks
FT = NCB * T             # 1024 mask free cols per core
NFT = FT // 512          # 2 f-tiles (512 = 8 n x 64 t)
NPF = 512 // T           # 8 n's per f-tile
NYT = YC // 128          # 8 y-tiles

MM_MODE = os.environ.get("SPIKE_MM_MODE", "f32rfix")  # f32rfix | f32r | bf16x2 | fp32
FIX_EPS = 4e-3  # f32rfix: host-recompute elements with |V-1| margin below this
TRACE = False

_cache = {}
LAST_RESULTS = None


def _ensure_ntff_hook():
    """Register the axon NTFF profiling hook if the environment lacks
    antenv.axon_hooks (the slim agent image) but has trn_agent_boot.
    Only adds capability; no-op when the real module exists."""
    try:
        import antenv.axon_hooks  # noqa: F401
        return
    except ImportError:
        pass
    try:
        import types
        from trn_agent_boot.trn_boot import _ntff_profile_via_ctypes
        hook = _ntff_profile_via_ctypes('/opt/axon/libaxon_pjrt.so')
        if hook is None:
            return
        import antenv
        mod = types.ModuleType('antenv.axon_hooks')
        mod.get_axon_ntff_profile_hook = lambda: hook
        mod.set_axon_ntff_profile_hook = lambda h: None
        sys.modules['antenv.axon_hooks'] = mod
        antenv.axon_hooks = mod
    except Exception:
        pass


def _safe_upload_artifacts():
    """upload_artifacts needs a bucket; make it degrade to a no-op path
    so tracing works in sandboxes without one."""
    try:
        from concourse import bass_utils
        orig = bass_utils.upload_artifacts
        if getattr(bass_utils, "_ul_wrapped", False):
            return
        def wrapped(tmpdir):
            try:
                return orig(tmpdir)
            except Exception:
                return str(tmpdir)
        bass_utils.upload_artifacts = wrapped
        bass_utils._ul_wrapped = True
    except Exception:
        pass


def _build_nc(reps=1):
    import concourse.bacc as bacc
    import concourse.mybir as mybir
    import concourse.tile as tile

    dt = mybir.dt
    f32 = dt.float32
    nc = bacc.Bacc("TRN2", target_bir_lowering=False, debug=False)

    if MM_MODE == "bf16x2":
        w_hi_d = nc.dram_tensor("w_hi", (X, YC), dt.bfloat16, kind="ExternalInput")
        w_lo_d = nc.dram_tensor("w_lo", (X, YC), dt.bfloat16, kind="ExternalInput")
    else:
        w_dt = dt.float32r if MM_MODE in ("f32r", "f32rfix") else f32
        w_d = nc.dram_tensor("w", (X, YC), w_dt, kind="ExternalInput")
    # aux = [inT (KC*NCB) | tb (T) | revt (T)] packed as one tensor so
    # startup needs a single DMA issue on the critical path
    aux_d = nc.dram_tensor("aux", (128, KC * NCB + 2 * T), f32,
                           kind="ExternalInput")
    out_d = nc.dram_tensor("out", (YC, NCB), f32, kind="ExternalOutput")
    if MM_MODE == "f32rfix":
        marg_d = nc.dram_tensor("marg", (YC, NCB), f32, kind="ExternalOutput")

    mask_dt = {"bf16x2": dt.bfloat16, "f32r": dt.float32r,
               "f32rfix": dt.float32r}.get(MM_MODE, f32)

    with tile.TileContext(nc) as tc:
        with tc.tile_pool(name="const", bufs=1) as cpool, \
             tc.tile_pool(name="wp", bufs=1) as wpool, \
             tc.tile_pool(name="mp", bufs=1) as mpool, \
             tc.tile_pool(name="ps", bufs=8, space="PSUM") as ps, \
             tc.tile_pool(name="sz", bufs=6) as szpool, \
             tc.tile_pool(name="sm", bufs=8) as smpool, \
             tc.tile_pool(name="po", bufs=4) as popool:
            neg1_sb = cpool.tile([128, 1], f32, tag="neg1")
            nc.vector.memset(neg1_sb, -1.0)

            # PE warmup: a few fp32 matmuls on junk data keep the PE busy
            # through the startup DMA window so HAM un-throttles (1.2 ->
            # 2.4 GHz) before the first real matmul arrives.
            junk_sb = cpool.tile([128, 512], f32, tag="junk")
            nc.gpsimd.memset(junk_sb, 1.0)
            warm_pt = ps.tile([128, 512], f32, tag="pt", name="warm_pt")
            for _ in range(2):
                nc.tensor.matmul(warm_pt, junk_sb[:, 0:128], junk_sb[:],
                                 start=True, stop=True)

            for rep in range(reps):
                aux_sb = cpool.tile([128, KC * NCB + 2 * T], f32, tag="aux")
                nc.sync.dma_start(out=aux_sb, in_=aux_d.ap())
                inT_sb = aux_sb[:, 0:KC * NCB]
                tb_sb = aux_sb[:, KC * NCB:KC * NCB + T]
                revt_sb = aux_sb[:, KC * NCB + T:KC * NCB + 2 * T]

                # weight chunks, resident
                if MM_MODE == "bf16x2":
                    w_tiles = []
                    for k in range(KC):
                        th = wpool.tile([128, YC], dt.bfloat16, tag=f"wh{k}")
                        tl = wpool.tile([128, YC], dt.bfloat16, tag=f"wl{k}")
                        nc.sync.dma_start(out=th,
                                          in_=w_hi_d.ap()[k * 128:(k + 1) * 128, :])
                        nc.sync.dma_start(out=tl,
                                          in_=w_lo_d.ap()[k * 128:(k + 1) * 128, :])
                        w_tiles.append((th, tl))
                else:
                    w_tiles = []
                    for k in range(KC):
                        tw = wpool.tile([128, YC], w_dt, tag=f"w{k}")
                        nc.sync.dma_start(out=tw,
                                          in_=w_d.ap()[k * 128:(k + 1) * 128, :])
                        w_tiles.append(tw)

                mask_tiles = [mpool.tile([128, FT], mask_dt, tag=f"m{k}",
                                         name=f"mask{k}")
                              for k in range(KC)]
                rm_tiles = [smpool.tile([128, NCB], f32, tag="rm",
                                        name=f"rm{yt}")
                            for yt in range(NYT)]
                mg_tiles = [smpool.tile([128, NCB], f32, tag="mg",
                                        name=f"mg{yt}")
                            for yt in range(NYT)] if MM_MODE == "f32rfix" else None

                def emit_mask(k):
                    t_b = tb_sb.unsqueeze(1).broadcast_to((128, NCB, T))
                    s_b = inT_sb[:, k * NCB:(k + 1) * NCB].unsqueeze(2) \
                        .broadcast_to((128, NCB, T))
                    nc.vector.tensor_tensor(
                        mask_tiles[k][:].rearrange("p (n t) -> p n t", n=NCB),
                        t_b, s_b, mybir.AluOpType.is_ge)

                def emit_mm(pt, k, yt, f):
                    rhs = mask_tiles[k][:, f * 512:(f + 1) * 512]
                    if MM_MODE == "bf16x2":
                        th, tl = w_tiles[k]
                        nc.tensor.matmul(pt, th[:, yt * 128:(yt + 1) * 128], rhs,
                                         start=(k == 0), stop=False)
                        nc.tensor.matmul(pt, tl[:, yt * 128:(yt + 1) * 128], rhs,
                                         start=False, stop=(k == KC - 1))
                    else:
                        lhsT = w_tiles[k][:, yt * 128:(yt + 1) * 128]
                        nc.tensor.matmul(pt, lhsT, rhs,
                                         start=(k == 0), stop=(k == KC - 1))

                def emit_post(pt, yt, f):
                    s_t = szpool.tile([128, 512], f32, tag="s")
                    nc.scalar.activation(s_t, pt,
                                         mybir.ActivationFunctionType.Sign,
                                         bias=neg1_sb[:])
                    if mg_tiles is not None:
                        a_t = szpool.tile([128, 512], f32, tag="a")
                        nc.scalar.activation(a_t, pt,
                                             mybir.ActivationFunctionType.Abs,
                                             bias=neg1_sb[:])
                        nc.vector.tensor_reduce(
                            mg_tiles[yt][:, f * NPF:(f + 1) * NPF],
                            a_t[:].rearrange("p (n t) -> p n t", n=NPF),
                            axis=mybir.AxisListType.X, op=mybir.AluOpType.min)
                    z_t = szpool.tile([128, 512], f32, tag="z")
                    r_b = revt_sb.unsqueeze(1).broadcast_to((128, NPF, T))
                    nc.vector.tensor_tensor(
                        z_t[:].rearrange("p (n t) -> p n t", n=NPF),
                        s_t[:].rearrange("p (n t) -> p n t", n=NPF),
                        r_b, mybir.AluOpType.mult)
                    nc.vector.tensor_reduce(
                        rm_tiles[yt][:, f * NPF:(f + 1) * NPF],
                        z_t[:].rearrange("p (n t) -> p n t", n=NPF),
                        axis=mybir.AxisListType.X, op=mybir.AluOpType.max)

                # f0 pass: k-outer so the PE trails the mask builder
                # without stalling; all 8 banks accumulate in parallel.
                pts = []
                for k in range(KC):
                    emit_mask(k)
                    for yt in range(NYT):
                        if k == 0:
                            pts.append(ps.tile([128, 512], f32, tag="pt",
                                               name=f"pt0_{yt}"))
                        emit_mm(pts[yt], k, yt, 0)
                for yt in range(NYT):
                    emit_post(pts[yt], yt, 0)

                # f1 pass: masks all resident now -> y-outer so banks
                # finish staggered and postproc overlaps later y-tiles.
                for yt in range(NYT):
                    pt = ps.tile([128, 512], f32, tag="pt", name=f"pt1_{yt}")
                    for k in range(KC):
                        emit_mm(pt, k, yt, 1)
                    emit_post(pt, yt, 1)
                    tmp_t = popool.tile([128, NCB], f32, tag="tmp")
                    nc.scalar.activation(tmp_t, rm_tiles[yt],
                                         mybir.ActivationFunctionType.Relu,
                                         bias=neg1_sb[:])
                    out_t = popool.tile([128, NCB], f32, tag="pout")
                    nc.scalar.activation(out_t, tmp_t,
                                         mybir.ActivationFunctionType.Copy,
                                         bias=float(T), scale=-1.0)
                    nc.sync.dma_start(out=out_d.ap()[yt * 128:(yt + 1) * 128, :],
                                      in_=out_t)
                    if mg_tiles is not None:
                        nc.sync.dma_start(
                            out=marg_d.ap()[yt * 128:(yt + 1) * 128, :],
                            in_=mg_tiles[yt])

    nc.compile()
    return nc


def _make_in_maps(inputs):
    import ml_dtypes

    input = np.ascontiguousarray(np.asarray(inputs["input"], dtype=np.float32))
    weight = np.ascontiguousarray(np.asarray(inputs["weight"], dtype=np.float32))
    t_series = np.asarray(inputs["t_series"], dtype=np.float32).reshape(-1)

    TB = np.tile(t_series, (128, 1)).astype(np.float32)
    REVT = np.tile((np.float32(T) - np.arange(T, dtype=np.float32)), (128, 1))

    in_maps = []
    for c in range(8):
        yb, nb = c % Y_SH, c // Y_SH
        wsl = np.ascontiguousarray(weight[:, yb * YC:(yb + 1) * YC])
        insl = input[nb * NCB:(nb + 1) * NCB, :]          # (NCB, X)
        inT = insl.reshape(NCB, KC, 128).transpose(2, 1, 0).reshape(128, KC * NCB)
        aux = np.ascontiguousarray(
            np.concatenate([inT, TB, REVT], axis=1).astype(np.float32))
        m = {"aux": aux}
        if MM_MODE == "bf16x2":
            w_hi = wsl.astype(ml_dtypes.bfloat16)
            w_lo = (wsl - w_hi.astype(np.float32)).astype(ml_dtypes.bfloat16)
            m["w_hi"] = w_hi
            m["w_lo"] = w_lo
        else:
            m["w"] = wsl
        in_maps.append(m)
    return in_maps


def kernel(input, weight, t_series, T=64, **unused):
    global LAST_RESULTS
    from concourse import bass_utils

    _ensure_ntff_hook()
    _safe_upload_artifacts()
    if "nc" not in _cache:
        _cache["nc"] = _build_nc()
    nc = _cache["nc"]

    _cache["t_series"] = np.asarray(t_series, dtype=np.float32).reshape(-1)
    in_maps = _make_in_maps(
        {"input": input, "weight": weight, "t_series": t_series})

    res = bass_utils.run_bass_kernel_spmd(
        nc, in_maps, core_ids=list(range(8)), trace=TRACE)
    LAST_RESULTS = res

    O = np.empty((YY, NN), dtype=np.float32)
    for c, r in enumerate(res.results):
        yb, nb = c % Y_SH, c // Y_SH
        O[yb * YC:(yb + 1) * YC, nb * NCB:(nb + 1) * NCB] = r["out"]
    out = np.ascontiguousarray(O.T)

    if MM_MODE == "f32rfix":
        M = np.empty((YY, NN), dtype=np.float32)
        for c, r in enumerate(res.results):
            yb, nb = c % Y_SH, c // Y_SH
            M[yb * YC:(yb + 1) * YC, nb * NCB:(nb + 1) * NCB] = r["marg"]
        _host_fixup(out, M.T, np.asarray(input, np.float32),
                    np.asarray(weight, np.float32))
    return out


def _host_fixup(out, margin, input, weight):
    """Recompute exactly (fp64) every element whose f32r |V-1| margin is
    within the f32r matmul error bound; in-place on `out`."""
    flags = margin < FIX_EPS
    if not flags.any():
        return
    # first step index j with t_series[j] >= in; == T means never spikes
    s = np.searchsorted(_cache.get("t_series", np.arange(T, dtype=np.float32)),
                        input, side="left").astype(np.int64)
    s = np.clip(s, 0, T)
    w64 = weight.astype(np.float64)
    for n in np.unique(np.nonzero(flags)[0]):
        ys = np.nonzero(flags[n])[0]
        d = np.zeros((T + 1, len(ys)))
        np.add.at(d, s[n], w64[:, ys])           # scatter rows by spike step
        V = np.cumsum(d[:T], axis=0)
        c = V > 1.0
        any_c = c.any(axis=0)
        idx = np.argmax(c, axis=0)
        out[n, ys] = np.where(any_c, idx + 1, T).astype(np.float32)



# revision 6
# speedup vs baseline: 1.0449x; 1.0449x over previous
"""Trainium2 Bass kernel for the spiking-dense first-crossing problem.

Computes out[n,y] = min(1 + argmax_t(V[t,n,y] > 1), 64) where
V[t] = (spike mask up to t) @ weight, via one big masked matmul:

  V^T[(y), (n,t)] = W_slice^T @ mask   (W stationary, y on PSUM partitions)

All-bf16 datapath: spike times are pre-ceiled on host so they are exact
integers in bf16 (mask compare unchanged), weight is rounded to bf16 and
any element whose |V-1| margin is below FIX_EPS is recomputed exactly on
host from the full-precision weight (same margin-fixup scheme as the
f32r variant, slightly larger eps).

First-crossing extraction per PSUM bank: one DVE scalar_tensor_tensor
z = (V > 1) * (T - t), reduce_max -> rm, out = 65 - rm (clipped to 64).
Margin: ACT |V-1| -> Pool (gpsimd) reduce_min.

Sharding: 2-way over Y (output cols) x 4-way over batch N across the 8
NeuronCores; each core computes a (1024 y, 16 n) block of out^T. The
full weight column-slice (2048 x 1024, bf16) stays resident in SBUF.
Chunk-0's mask ships pre-built from host inside the aux tensor so the
first real matmul is not gated on the on-chip mask builder.
"""
import os
import sys
import numpy as np

for _p in ('/opt/trn_rl_repo',):
    if os.path.isdir(_p) and _p not in sys.path:
        sys.path.append(_p)

X, T, NN, YY = 2048, 64, 64, 2048
Y_SH, N_SH = 2, 4
YC = YY // Y_SH          # 1024 y-cols per core
NCB = NN // N_SH         # 16 batch rows per core
KC = X // 128            # 16 contraction chunks
FT = NCB * T             # 1024 mask free cols per core
NFT = FT // 512          # 2 f-tiles (512 = 8 n x 64 t)
NPF = 512 // T           # 8 n's per f-tile
NYT = YC // 128          # 8 y-tiles

# aux column layout (bf16): [inT (KC*NCB) | tb (T) | revt (T) | mask0 (FT)]
AUXC = KC * NCB + 2 * T + FT

FIX_EPS = 7e-3  # host-recompute elements with |V-1| margin below this
TRACE = False

_cache = {}
LAST_RESULTS = None


def _ensure_ntff_hook():
    """Register the axon NTFF profiling hook if the environment lacks
    antenv.axon_hooks (the slim agent image) but has trn_agent_boot.
    Only adds capability; no-op when the real module exists."""
    try:
        import antenv.axon_hooks  # noqa: F401
        return
    except ImportError:
        pass
    try:
        import types
        from trn_agent_boot.trn_boot import _ntff_profile_via_ctypes
        hook = _ntff_profile_via_ctypes('/opt/axon/libaxon_pjrt.so')
        if hook is None:
            return
        import antenv
        mod = types.ModuleType('antenv.axon_hooks')
        mod.get_axon_ntff_profile_hook = lambda: hook
        mod.set_axon_ntff_profile_hook = lambda h: None
        sys.modules['antenv.axon_hooks'] = mod
        antenv.axon_hooks = mod
    except Exception:
        pass


def _safe_upload_artifacts():
    """upload_artifacts needs a bucket; make it degrade to a no-op path
    so tracing works in sandboxes without one."""
    try:
        from concourse import bass_utils
        orig = bass_utils.upload_artifacts
        if getattr(bass_utils, "_ul_wrapped", False):
            return
        def wrapped(tmpdir):
            try:
                return orig(tmpdir)
            except Exception:
                return str(tmpdir)
        bass_utils.upload_artifacts = wrapped
        bass_utils._ul_wrapped = True
    except Exception:
        pass


def _build_nc(reps=1):
    import concourse.bacc as bacc
    import concourse.mybir as mybir
    import concourse.tile as tile

    dt = mybir.dt
    f32 = dt.float32
    bf16 = dt.bfloat16
    nc = bacc.Bacc("TRN2", target_bir_lowering=False, debug=False)

    w_d = nc.dram_tensor("w", (X, YC), bf16, kind="ExternalInput")
    aux_d = nc.dram_tensor("aux", (128, AUXC), bf16, kind="ExternalInput")
    out_d = nc.dram_tensor("out", (128, NYT * NCB), f32, kind="ExternalOutput")
    marg_d = nc.dram_tensor("marg", (128, NYT * NCB), f32,
                            kind="ExternalOutput")

    with tile.TileContext(nc) as tc:
        with tc.tile_pool(name="const", bufs=1) as cpool, \
             tc.tile_pool(name="wp", bufs=1) as wpool, \
             tc.tile_pool(name="mp", bufs=1) as mpool, \
             tc.tile_pool(name="ps", bufs=8, space="PSUM") as ps, \
             tc.tile_pool(name="sz", bufs=6) as szpool, \
             tc.tile_pool(name="po", bufs=1) as popool:
            # PE warmup: short bf16 matmuls on junk data keep the PE busy
            # through the startup DMA window so HAM un-throttles before
            # the first real matmul arrives.
            neg1_sb = cpool.tile([128, 1], f32, tag="neg1")
            nc.vector.memset(neg1_sb, -1.0)
            junk_sb = cpool.tile([128, 512], bf16, tag="junk")
            nc.gpsimd.memset(junk_sb, 1.0)
            warm_pt = ps.tile([128, 128], f32, tag="pt", name="warm_pt")
            for _ in range(10):
                nc.tensor.matmul(warm_pt, junk_sb[:, 0:128],
                                 junk_sb[:, 0:128], start=True, stop=True)

            for rep in range(reps):
                aux_sb = cpool.tile([128, AUXC], bf16, tag="aux")
                nc.sync.dma_start(out=aux_sb, in_=aux_d.ap())
                inT_sb = aux_sb[:, 0:KC * NCB]
                tb_sb = aux_sb[:, KC * NCB:KC * NCB + T]
                revt_sb = aux_sb[:, KC * NCB + T:KC * NCB + 2 * T]
                mask0_sb = aux_sb[:, KC * NCB + 2 * T:AUXC]

                # weight chunks, resident
                w_tiles = []
                for k in range(KC):
                    tw = wpool.tile([128, YC], bf16, tag=f"w{k}")
                    nc.sync.dma_start(out=tw,
                                      in_=w_d.ap()[k * 128:(k + 1) * 128, :])
                    w_tiles.append(tw)

                # mask chunk 0 arrives via aux; 1..15 built on DVE
                mask_tiles = [mask0_sb] + \
                    [mpool.tile([128, FT], bf16, tag=f"m{k}", name=f"mask{k}")
                     for k in range(1, KC)]
                rm_tiles = [szpool.tile([128, NCB], bf16, tag="rm",
                                        name=f"rm{yt}")
                            for yt in range(NYT)]
                out_sh = popool.tile([128, NYT * NCB], f32, tag="osh")
                mg_sh = popool.tile([128, NYT * NCB], f32, tag="msh")

                def emit_mask(k):
                    t_b = tb_sb.unsqueeze(1).broadcast_to((128, NCB, T))
                    s_b = inT_sb[:, k * NCB:(k + 1) * NCB].unsqueeze(2) \
                        .broadcast_to((128, NCB, T))
                    nc.vector.tensor_tensor(
                        mask_tiles[k][:].rearrange("p (n t) -> p n t", n=NCB),
                        t_b, s_b, mybir.AluOpType.is_ge)

                def emit_mm(pt, k, yt, f):
                    rhs = mask_tiles[k][:, f * 512:(f + 1) * 512]
                    lhsT = w_tiles[k][:, yt * 128:(yt + 1) * 128]
                    nc.tensor.matmul(pt, lhsT, rhs,
                                     start=(k == 0), stop=(k == KC - 1))

                def emit_post(pt, yt, f):
                    # rm = max_t (V > 1) * (T - t); 0 when never crossed
                    z_t = szpool.tile([128, 512], bf16, tag="z")
                    r_b = revt_sb.unsqueeze(1).broadcast_to((128, NPF, T))
                    nc.vector.scalar_tensor_tensor(
                        z_t[:].rearrange("p (n t) -> p n t", n=NPF),
                        pt[:].rearrange("p (n t) -> p n t", n=NPF),
                        1.0, r_b,
                        mybir.AluOpType.is_gt, mybir.AluOpType.mult)
                    nc.vector.tensor_reduce(
                        rm_tiles[yt][:, f * NPF:(f + 1) * NPF],
                        z_t[:].rearrange("p (n t) -> p n t", n=NPF),
                        axis=mybir.AxisListType.X, op=mybir.AluOpType.max)
                    # margin = min_t |V - 1| on ACT + Pool
                    a_t = szpool.tile([128, 512], f32, tag="a")
                    nc.scalar.activation(a_t, pt,
                                         mybir.ActivationFunctionType.Abs,
                                         bias=neg1_sb[:])
                    nc.vector.tensor_reduce(
                        mg_sh[:, yt * NCB + f * NPF:yt * NCB + (f + 1) * NPF],
                        a_t[:].rearrange("p (n t) -> p n t", n=NPF),
                        axis=mybir.AxisListType.X, op=mybir.AluOpType.min)

                # f0 pass: k-outer so the PE trails the mask builder
                # without stalling; all 8 banks accumulate in parallel.
                pts = []
                for k in range(KC):
                    if k > 0:
                        emit_mask(k)
                    for yt in range(NYT):
                        if k == 0:
                            pts.append(ps.tile([128, 512], f32, tag="pt",
                                               name=f"pt0_{yt}"))
                        emit_mm(pts[yt], k, yt, 0)
                for yt in range(NYT):
                    emit_post(pts[yt], yt, 0)

                # f1 pass: masks all resident now -> y-outer so banks
                # finish staggered and postproc overlaps later y-tiles.
                for yt in range(NYT):
                    pt = ps.tile([128, 512], f32, tag="pt", name=f"pt1_{yt}")
                    for k in range(KC):
                        emit_mm(pt, k, yt, 1)
                    emit_post(pt, yt, 1)
                    # out = 65 - rm  (crossed at t -> t+1; never -> 65)
                    nc.scalar.activation(
                        out_sh[:, yt * NCB:(yt + 1) * NCB], rm_tiles[yt],
                        mybir.ActivationFunctionType.Copy,
                        bias=float(T + 1), scale=-1.0)
                # clip the never-crossed 65s to 64 in one pass
                nc.vector.tensor_scalar_min(out_sh[:], out_sh[:], float(T))
                nc.sync.dma_start(out=out_d.ap(), in_=out_sh)
                nc.sync.dma_start(out=marg_d.ap(), in_=mg_sh)

    nc.compile()
    return nc


def _make_in_maps(inputs):
    import ml_dtypes

    input = np.ascontiguousarray(np.asarray(inputs["input"], dtype=np.float32))
    weight = np.ascontiguousarray(np.asarray(inputs["weight"], dtype=np.float32))
    t_series = np.asarray(inputs["t_series"], dtype=np.float32).reshape(-1)

    s_ceil = np.ceil(input).astype(np.float32)   # exact in bf16 (ints <= 64)
    TB = np.tile(t_series, (128, 1)).astype(np.float32)
    REVT = np.tile((np.float32(T) - np.arange(T, dtype=np.float32)), (128, 1))
    tgrid = np.arange(T, dtype=np.float32)

    in_maps = []
    mask0_cache = {}
    for c in range(8):
        yb, nb = c % Y_SH, c // Y_SH
        wsl = np.ascontiguousarray(
            weight[:, yb * YC:(yb + 1) * YC]).astype(ml_dtypes.bfloat16)
        scl = s_ceil[nb * NCB:(nb + 1) * NCB, :]          # (NCB, X)
        inT = scl.reshape(NCB, KC, 128).transpose(2, 1, 0).reshape(128, KC * NCB)
        if nb not in mask0_cache:
            # chunk-0 mask, host-built: mask0[p, n*T+t] = t >= ceil(s[n, p])
            m0 = tgrid[None, None, :] >= scl[:, 0:128][:, :, None]  # (n,p,t)
            mask0_cache[nb] = np.ascontiguousarray(
                m0.transpose(1, 0, 2).reshape(128, FT).astype(np.float32))
        aux = np.ascontiguousarray(
            np.concatenate([inT, TB, REVT, mask0_cache[nb]], axis=1)
        ).astype(ml_dtypes.bfloat16)
        in_maps.append({"aux": aux, "w": wsl})
    return in_maps


def kernel(input, weight, t_series, T=64, **unused):
    global LAST_RESULTS
    from concourse import bass_utils

    _ensure_ntff_hook()
    _safe_upload_artifacts()
    if "nc" not in _cache:
        _cache["nc"] = _build_nc()
    nc = _cache["nc"]

    _cache["t_series"] = np.asarray(t_series, dtype=np.float32).reshape(-1)
    in_maps = _make_in_maps(
        {"input": input, "weight": weight, "t_series": t_series})

    res = bass_utils.run_bass_kernel_spmd(
        nc, in_maps, core_ids=list(range(8)), trace=TRACE)
    LAST_RESULTS = res

    # device layout: out[p, yt*NCB + n] = result for y = yt*128+p, batch n
    O = np.empty((YY, NN), dtype=np.float32)
    M = np.empty((YY, NN), dtype=np.float32)
    for c, r in enumerate(res.results):
        yb, nb = c % Y_SH, c // Y_SH
        ob = r["out"].reshape(128, NYT, NCB).transpose(1, 0, 2).reshape(YC, NCB)
        mb = r["marg"].reshape(128, NYT, NCB).transpose(1, 0, 2).reshape(YC, NCB)
        O[yb * YC:(yb + 1) * YC, nb * NCB:(nb + 1) * NCB] = ob
        M[yb * YC:(yb + 1) * YC, nb * NCB:(nb + 1) * NCB] = mb
    out = np.ascontiguousarray(O.T)

    _host_fixup(out, M.T, np.asarray(input, np.float32),
                np.asarray(weight, np.float32))
    return out


def _host_fixup(out, margin, input, weight):
    """Recompute exactly (fp64) every element whose bf16 |V-1| margin is
    within the bf16 matmul error bound; in-place on `out`."""
    flags = margin < FIX_EPS
    if not flags.any():
        return
    # first step index j with t_series[j] >= in; == T means never spikes
    s = np.searchsorted(_cache.get("t_series", np.arange(T, dtype=np.float32)),
                        input, side="left").astype(np.int64)
    s = np.clip(s, 0, T)
    w64 = weight.astype(np.float64)
    for n in np.unique(np.nonzero(flags)[0]):
        ys = np.nonzero(flags[n])[0]
        d = np.zeros((T + 1, len(ys)))
        np.add.at(d, s[n], w64[:, ys])           # scatter rows by spike step
        V = np.cumsum(d[:T], axis=0)
        c = V > 1.0
        any_c = c.any(axis=0)
        idx = np.argmax(c, axis=0)
        out[n, ys] = np.where(any_c, idx + 1, T).astype(np.float32)
